# revision 1
# baseline (speedup 1.0000x reference)
"""Trainium2 SPMD kernel for nn_AutoregressiveDecoder (gnn_message_passing).

Math (reference, per context g in 0..N-1, N=384):
    h1[g]  = concat(z, e_g) @ W1 = H0 + e_g (x) W1r     # H0 = z @ W1[:128]
    A[g]   = relu(P_g @ h1[g])         P_g = partials[g]
    h2[g]  = A[g] @ W2
    h3[g]  = P_g @ h2[g]
    S[g,:] = h3[g][g,:] @ h3[g].T      (row g of supplement, pre-tril)
    out    = x + 0.5*(tril(S) + tril(S).T)

8 cores x 48 contexts, raw Bass (manual semaphores), fp32r matmuls except
the A@W2 stage in bf16.  Per context i (software-pipelined, skew 3):
    mm1  A_T[h,:]  = sum_j H0[j,h] Pt[j,:]  (+ rank-1 W1r (x) pcol)  N=384
    mm2  h2[j,k]   = sum_h A_T[h,j] W2[h,k]                          N=128 bf16
    mm3  h3T[k,:]  = sum_j h2[j,k] PtAug[j,:]  (col 384 = d vector)  N=385
    mm4  S[1,:]    = sum_k d[k] h3T[k,:]   (psum row aliased in h3ps) N=384
Pt = P_g.T pre-transposed on host; PtAug's col 384 is P_g[g,:] so mm3 also
yields d = h3[g][g,:].  tril/symmetrize/(+x) happen on host at unshard.
PE stream at iter i: mm1(i), mm2(i-1), mm3(i-2), mm4(i-3) -- the ACT/DVE
relu/copies of stage k run a full iteration before their PE consumer.
"""

import os
from contextlib import ExitStack

import numpy as np
import ml_dtypes

import concourse.bass as bass
import concourse.mybir as mybir
from concourse.bass_utils import run_bass_kernel_spmd

N = 384
D = 128
HID = 256
HID2 = 128
NCORES = 8
NB = N // NCORES  # 48 contexts per core
W = N + 2  # pt width: prow column at N, plus even-size pad (fp32r dst rule)
PTBUF = 8  # pt SBUF ring depth
SRBUF = 8  # S-row SBUF ring depth

F32 = mybir.dt.float32
F32R = mybir.dt.float32r
BF16 = mybir.dt.bfloat16
AFT = mybir.ActivationFunctionType

_NC_CACHE = {}
LAST_RESULT = None  # test.py reads exec_time_ns from here


def _round_f32r(a: np.ndarray) -> np.ndarray:
    """Round fp32 to fp32r (TF32-like: low 12 mantissa bits cleared, RNE)."""
    u = np.ascontiguousarray(a, dtype=np.float32).view(np.uint32)
    add = np.uint32(0x7FF) + ((u >> np.uint32(12)) & np.uint32(1))
    r = (u + add) & np.uint32(0xFFFFF000)
    return r.view(np.float32)


def _build_nc() -> bass.Bass:
    nc = bass.Bass()
    pt_d = nc.declare_dram_parameter("pt", [NB, 128, 3 * W], BF16, isOutput=False)
    pcol_d = nc.declare_dram_parameter("pcol", [1, NB * N], BF16, isOutput=False)
    h0f_d = nc.declare_dram_parameter("h0f", [128, 3 * HID], BF16, isOutput=False)
    w1r_d = nc.declare_dram_parameter("w1r", [1, HID], BF16, isOutput=False)
    w2f_d = nc.declare_dram_parameter("w2f", [128, 2 * HID2], BF16, isOutput=False)
    out_ds = [
        nc.declare_dram_parameter(f"o{b:02d}", [1, N], F32, isOutput=True)
        for b in range(NB)
    ]

    ctx = ExitStack()
    with ctx:
        # ---- persistent SBUF ----
        h0f = ctx.enter_context(nc.sbuf_tensor("h0f_s", [128, 3 * HID], BF16))
        w1r = ctx.enter_context(nc.sbuf_tensor("w1r_s", [1, HID], BF16))
        w2f = ctx.enter_context(nc.sbuf_tensor("w2f_s", [128, 2 * HID2], BF16))
        pcall = ctx.enter_context(nc.sbuf_tensor("pcall_s", [1, NB * N], BF16))
        pt = [
            ctx.enter_context(nc.sbuf_tensor(f"ptb{s}", [128, 3 * W], BF16))
            for s in range(PTBUF)
        ]
        at = [
            ctx.enter_context(nc.sbuf_tensor(f"atb{s}", [128, 2 * N], BF16))
            for s in range(3)
        ]
        h2sb = [
            ctx.enter_context(nc.sbuf_tensor(f"h2b{s}", [128, N], BF16))
            for s in range(3)
        ]
        h3sb = [
            ctx.enter_context(nc.sbuf_tensor(f"h3b{s}", [128, W], BF16))
            for s in range(3)
        ]
        srow = [
            ctx.enter_context(nc.sbuf_tensor(f"srowb{s}", [1, N], F32))
            for s in range(SRBUF)
        ]
        # ---- PSUM: 8 banks exactly ----
        aps = [
            [
                ctx.enter_context(
                    nc.psum_tensor(f"apsb{p}{h}", [128, N], F32)
                )
                for h in range(2)
            ]
            for p in range(2)
        ]  # aps[pair][hc]
        h2ps = [
            ctx.enter_context(nc.psum_tensor(f"h2psb{s}", [128, N], F32))
            for s in range(2)
        ]
        h3ps = [
            ctx.enter_context(nc.psum_tensor(f"h3psb{s}", [128, W], F32))
            for s in range(2)
        ]

        # ---- semaphores ----
        sem_const = ctx.enter_context(nc.semaphore("sem_const"))
        sem_pc2 = ctx.enter_context(nc.semaphore("sem_pc2"))
        sem_w2 = ctx.enter_context(nc.semaphore("sem_w2"))
        sem_pt = [
            ctx.enter_context(nc.semaphore(f"sem_pt{s}")) for s in range(PTBUF)
        ]
        sem_out = [
            ctx.enter_context(nc.semaphore(f"sem_out{s}")) for s in range(SRBUF)
        ]
        sem_mm1 = ctx.enter_context(nc.semaphore("sem_mm1"))
        sem_relu = ctx.enter_context(nc.semaphore("sem_relu"))
        sem_mm2 = ctx.enter_context(nc.semaphore("sem_mm2"))
        sem_h2c = ctx.enter_context(nc.semaphore("sem_h2c"))
        sem_mm3 = ctx.enter_context(nc.semaphore("sem_mm3"))
        sem_h3c = ctx.enter_context(nc.semaphore("sem_h3c"))
        sem_mm4 = ctx.enter_context(nc.semaphore("sem_mm4"))
        sem_sc = ctx.enter_context(nc.semaphore("sem_sc"))

        block = ctx.enter_context(nc.Block())

        NI = NB + 3  # pipeline iterations (skew 3)

        PCA = 8  # contexts whose pcol rows load before the loop starts

        @block.sync
        def _(sync):
            sync.dma_start(h0f[:, :], h0f_d[:, :]).then_inc(sem_const, 16)
            sync.dma_start(w1r[:, :], w1r_d[:, :]).then_inc(sem_const, 16)
            sync.dma_start(
                pcall[:, 0 : PCA * N], pcol_d[:, 0 : PCA * N]
            ).then_inc(sem_const, 16)
            sync.dma_start(w2f[:, 0:HID2], w2f_d[:, 0:HID2]).then_inc(sem_w2, 16)
            for i in range(NI):
                k = i - 3
                if 0 <= k < NB:
                    sync.wait_ge(sem_sc, k + 1)
                    sync.dma_start(
                        out_ds[k][:, :], srow[k % SRBUF][:, :]
                    ).then_inc(sem_out[k % SRBUF], 16)



        @block.gpsimd
        def _(g):
            for p in range(min(PTBUF, NB)):
                if p >= 2:
                    # keep only 2 prefetch DMAs in flight so pt(0) is not
                    # bandwidth-shared 6 ways (rings interleave packets)
                    g.wait_ge(sem_pt[p - 2], 16)
                g.dma_start(pt[p][:, :], pt_d[p]).then_inc(sem_pt[p], 16)
            for i in range(NI):
                p = i + PTBUF
                if p < NB:
                    g.wait_ge(sem_mm3, i + 1)
                    g.dma_start(
                        pt[p % PTBUF][:, :], pt_d[p]
                    ).then_inc(sem_pt[p % PTBUF], 16)


        @block.tensor
        def _(te):
            te.wait_ge(sem_const, 48)
            for i in range(NI):
                if i == 1:
                    te.wait_ge(sem_w2, 32)
                if i == PCA:
                    te.wait_ge(sem_pc2, 16)
                # ---- mm1(i): A_T chunks + rank-1, fp32r N=384 ----
                if i < NB:
                    # aps-pair-reuse wait (sem_relu >= 2i-2) is implied by the
                    # previous iteration's wait before mm2.
                    te.wait_ge(sem_pt[i % PTBUF], 16 * (i // PTBUF + 1))
                    ptt = pt[i % PTBUF]
                    for hc in range(2):
                        for t in range(3):
                            nc.tensor.matmul(
                                aps[i % 2][hc][:, :],
                                h0f[:, t * HID + hc * 128 : t * HID + hc * 128 + 128],
                                ptt[:, t * W : t * W + N],
                                start=(t == 0),
                                stop=False,
                                skip_group_check=True,
                            )

                # ---- mm2(i-1): h2 = A@W2, bf16 N=128 ----
                k = i - 1
                if 0 <= k < NB:
                    te.wait_ge(sem_relu, 2 * k + 2)
                    # h2ps[k%2]-reuse wait is implied by the previous
                    # iteration's wait before mm3.
                    dst = h2ps[k % 2]
                    for jc in range(3):
                        for ht in range(2):
                            mm = nc.tensor.matmul(
                                dst[:, jc * 128 : (jc + 1) * 128],
                                at[k % 3][
                                    :, ht * N + jc * 128 : ht * N + jc * 128 + 128
                                ],
                                w2f[:, ht * HID2 : (ht + 1) * HID2],
                                start=(ht == 0),
                                stop=(ht == 1),
                            )
                    if not (0 <= i - 2 < NB):
                        mm.then_inc(sem_mm2, 1)  # no mm3 rider this iter
                # ---- rank-1 pair for mm1(i), emitted after the short bf16
                # matmuls: the K=1 weight loads can't prefetch past a
                # full-height in-flight matmul, so placing them here turns two
                # ~250ns stalls into one small one ----
                if i < NB:
                    r1 = []
                    for hc in range(2):
                        r1.append(
                            nc.tensor.matmul(
                                aps[i % 2][hc][:, :],
                                w1r[:, hc * 128 : (hc + 1) * 128],
                                pcall[:, i * N : (i + 1) * N],
                                start=False,
                                stop=True,
                                skip_group_check=True,
                            )
                        )
                    if i < 2:
                        r1[0].then_inc(sem_mm1, 1)
                        r1[1].then_inc(sem_mm1, 1)
                    else:
                        r1[1].then_inc(sem_mm1, 1)  # hc0 group drained
                        # hc1's inc rides on mm3(i-2)-t1 below
                # ---- mm3(i-2): h3T (+d col), N=386 ----
                k = i - 2
                if 0 <= k < NB:
                    te.wait_ge(sem_h2c, k + 1)
                    if k >= 2:
                        # h3ps[k%2]-reuse is implied by last iter's mm4 wait
                        te.wait_ge(sem_sc, k - 1)  # aliased S row was drained
                    dst = h3ps[k % 2]
                    ptt = pt[k % PTBUF]
                    for t in range(3):
                        mm = nc.tensor.matmul(
                            dst[:, :],
                            h2sb[k % 3][:, t * 128 : (t + 1) * 128],
                            ptt[:, t * W : (t + 1) * W],
                            start=(t == 0),
                            stop=(t == 2),
                        )
                        if t == 0 and k + 1 < NB:
                            # completion implies same-iter mm2(k+1) drained
                            mm.then_inc(sem_mm2, 1)
                        if t == 1 and i < NB:
                            # completion implies same-iter rank-1 hc1 drained
                            mm.then_inc(sem_mm1, 1)
                    mm.then_inc(sem_mm3, 1)
                # ---- mm4(i-3): S row into h3ps[k%2] partition 0 ----
                k = i - 3
                if 0 <= k < NB:
                    te.wait_ge(sem_h3c, k + 1)
                    mm = nc.tensor.matmul(
                        h3ps[k % 2][0:1, 0:N],
                        h3sb[k % 3][:, N : N + 1],
                        h3sb[k % 3][:, 0:N],
                        start=True,
                        stop=True,
                    )
                    mm.then_inc(sem_mm4, 1)

        @block.scalar
        def _(sc):
            sc.dma_start(w2f[:, HID2:], w2f_d[:, HID2:]).then_inc(sem_w2, 16)
            sc.dma_start(
                pcall[:, PCA * N :], pcol_d[:, PCA * N :]
            ).then_inc(sem_pc2, 16)
            for i in range(NI):
                k = i
                if k < NB:
                    if k >= 3:
                        sc.wait_ge(sem_mm2, k - 2)  # at[k%3] reuse
                    for hc in range(2):
                        sc.wait_ge(sem_mm1, 2 * k + hc + 1)
                        nc.scalar.activation(
                            at[k % 3][:, hc * N : (hc + 1) * N],
                            aps[k % 2][hc][:, :],
                            AFT.Relu,
                        ).then_inc(sem_relu, 1)


        @block.vector
        def _(ve):
            for i in range(NI):
                k = i - 1
                if 0 <= k < NB:
                    if k >= 3:
                        ve.wait_ge(sem_mm3, k - 2)  # h2sb[k%3] reuse
                    ve.wait_ge(sem_mm2, k + 1)
                    nc.vector.tensor_copy(
                        h2sb[k % 3][:, :], h2ps[k % 2][:, :]
                    ).then_inc(sem_h2c, 1)
                k = i - 2
                if 0 <= k < NB:
                    if k >= 3:
                        ve.wait_ge(sem_mm4, k - 2)  # h3sb[k%3] reuse
                    ve.wait_ge(sem_mm3, k + 1)
                    nc.vector.tensor_copy(
                        h3sb[k % 3][:, :], h3ps[k % 2][:, :]
                    ).then_inc(sem_h3c, 1)
                k = i - 3
                if 0 <= k < NB:
                    ve.wait_ge(sem_mm4, k + 1)
                    if k >= SRBUF:
                        ve.wait_ge(sem_out[k % SRBUF], 16 * (k // SRBUF))
                    nc.vector.tensor_copy(
                        srow[k % SRBUF][:, :], h3ps[k % 2][0:1, 0:N]
                    ).then_inc(sem_sc, 1)

    return nc


def _get_nc() -> bass.Bass:
    if "nc" not in _NC_CACHE:
        _NC_CACHE["nc"] = _build_nc()
    return _NC_CACHE["nc"]


def kernel(z, x, partials, W1, W2):
    global LAST_RESULT
    z = np.asarray(z, dtype=np.float32)
    x = np.asarray(x, dtype=np.float32)
    partials = np.asarray(partials, dtype=np.float32)
    W1 = np.asarray(W1, dtype=np.float32)
    W2 = np.asarray(W2, dtype=np.float32)

    H0 = z[0] @ W1[:D]  # [384, 256]
    h0f = (
        np.ascontiguousarray(H0.reshape(3, 128, HID).transpose(1, 0, 2))
        .reshape(128, 3 * HID)
        .astype(ml_dtypes.bfloat16)
    )
    w1r = np.ascontiguousarray(W1[D : D + 1]).astype(ml_dtypes.bfloat16)
    w2f = (
        np.ascontiguousarray(W2.reshape(2, 128, HID2).transpose(1, 0, 2))
        .reshape(128, 2 * HID2)
        .astype(ml_dtypes.bfloat16)
    )

    ptT = np.ascontiguousarray(partials.transpose(0, 2, 1))  # ptT[g,j,i]=P_g[i,j]
    ar = np.arange(N)
    prow = partials[ar, ar, :]  # [384, 384]  P_g[g, :]
    pcol = ptT[ar, ar, :]  # [384, 384]  P_g[:, g]

    in_maps = []
    for c in range(NCORES):
        gs = slice(c * NB, (c + 1) * NB)
        aug = np.zeros((NB, 3, 128, W), dtype=ml_dtypes.bfloat16)
        aug[..., :N] = ptT[gs].reshape(NB, 3, 128, N).astype(ml_dtypes.bfloat16)
        aug[..., N] = prow[gs].reshape(NB, 3, 128).astype(ml_dtypes.bfloat16)
        aug = np.ascontiguousarray(aug.transpose(0, 2, 1, 3)).reshape(NB, 128, 3 * W)
        in_maps.append(
            {
                "pt": aug,
                "pcol": np.ascontiguousarray(pcol[gs])
                .astype(ml_dtypes.bfloat16)
                .reshape(1, NB * N),
                "h0f": h0f,
                "w1r": w1r,
                "w2f": w2f,
            }
        )

    nc = _get_nc()
    res = run_bass_kernel_spmd(
        nc,
        in_maps,
        core_ids=list(range(NCORES)),
        trace=bool(os.environ.get("KERNEL_TRACE")),
    )
    LAST_RESULT = res
    S = np.concatenate(
        [
            np.concatenate(
                [
                    np.asarray(res.results[c][f"o{b:02d}"], np.float32)
                    for b in range(NB)
                ],
                axis=0,
            )
            for c in range(NCORES)
        ],
        axis=0,
    )  # [384, 384] raw supplement rows
    sup = np.tril(S)
    sup = (sup + sup.T) * np.float32(0.5)
    return (x + sup).astype(np.float32)



# revision 7
# speedup vs baseline: 1.3117x; 1.3117x over previous
"""Trainium2 SPMD kernel for nn_AutoregressiveDecoder (gnn_message_passing).

Math (reference, per context g in 0..N-1, N=384):
    h1[g]  = concat(z, e_g) @ W1                        # = H0 + e_g (x) W1r
    A[g]   = relu(P_g @ h1[g])         P_g = partials[g]
    h2[g]  = A[g] @ W2
    h3[g]  = P_g @ h2[g]
    S[g,:] = h3[g][g,:] @ h3[g].T      (row g of supplement, pre-tril)
    out    = x + 0.5*(tril(S) + tril(S).T)

8 cores x 48 contexts, interleaved assignment g = 8b + c (slot b on core c)
so the tril truncation width 8b+8 >= g+1 is static in the shared program.
All matmuls bf16 (PSUM f32). Per slot b (software-pipelined, skew 3):
    mm1  A_T[h,:]  = sum_j H0m[j,h] Pt[j,:]   F=384 x6   (rank-1 update of
         row g is folded into a per-slot copy of H0's chunk b//16 -- no
         separate K=1 matmuls, no pcol DMA)
    mm2  h2[j,k]   = sum_h A_T[h,j] W2[h,k]   F=128 x6
    mm3  h3T[k,:]  = sum_j h2[j,k] PtAug[j,:] F=8b+10 x3 (col 0 = prow ->
         h3T[:,0] = d = h3[g][g,:]; cols 1.. = h3T[:,j<8b+9])
    mm4  S[1,:]    = sum_k d[k] h3T[k,1:]     F=8b+8  (psum row aliased)
tril/symmetrize/(+x) happen on host at unshard.
PE stream at iter i: mm1(i), mm2(i-1), mm3(i-2), mm4(i-3); ACT does the
relus, DVE does S-row drain + h2/h3 PSUM->SBUF copies (single ordered
semaphore so the PE needs only ~3 waits per iteration).
"""

import os
from contextlib import ExitStack

import numpy as np
import ml_dtypes

import concourse.bass as bass
import concourse.mybir as mybir
from concourse.bass_utils import run_bass_kernel_spmd

N = 384
D = 128
HID = 256
HID2 = 128
NCORES = 8
NB = N // NCORES  # 48 contexts per core
W = N + 2  # pt tile chunk width: col 0 = prow, cols 1..384 = Pt, col 385 pad
PTBUF = 8  # pt SBUF ring depth
SRBUF = 8  # S-row SBUF ring depth

F32 = mybir.dt.float32
BF16 = mybir.dt.bfloat16
AFT = mybir.ActivationFunctionType

_NC_CACHE = {}
LAST_RESULT = None  # test.py reads exec_time_ns from here


def _jw(b):
    """Truncated S-row width for slot b: covers j <= g for all g = 8b+c."""
    return 8 * b + 8


def _h0m_piece(b):
    """h0m DMA piece index covering slot b (piece 0 = slot 0 alone so the
    first mm1 is gated on a 512B/partition DMA, not the full 24KB)."""
    return 0 if b == 0 else 1 if b < 8 else 1 + b // 8


N_H0M_PIECES = 7


def _build_nc() -> bass.Bass:
    nc = bass.Bass()
    pt_d = nc.declare_dram_parameter("pt", [NB, 128, 3 * W], BF16, isOutput=False)
    h0f_d = nc.declare_dram_parameter("h0f", [128, 3 * HID], BF16, isOutput=False)
    h0m_d = nc.declare_dram_parameter("h0m", [128, NB * HID], BF16, isOutput=False)
    w2f_d = nc.declare_dram_parameter("w2f", [128, 2 * HID2], BF16, isOutput=False)
    out_ds = [
        nc.declare_dram_parameter(f"o{b:02d}", [1, _jw(b)], F32, isOutput=True)
        for b in range(NB)
    ]

    NI = NB + 3  # PE pipeline iterations (skew 3)

    # ---- DVE op-order counter: ops per DVE iter i are sc(i-4), h2c(i-1),
    # h3c(i-2); a single semaphore counts them so consumers wait once. ----
    cnt = 0
    c_sc = {}
    c_h2c = {}
    c_h3c = {}
    for i in range(NB + 4):
        if 0 <= i - 4 < NB:
            cnt += 1
            c_sc[i - 4] = cnt
        if 0 <= i - 1 < NB:
            cnt += 1
            c_h2c[i - 1] = cnt
        if 0 <= i - 2 < NB:
            cnt += 1
            c_h3c[i - 2] = cnt

    ctx = ExitStack()
    with ctx:
        # ---- persistent SBUF ----
        h0f = ctx.enter_context(nc.sbuf_tensor("h0f_s", [128, 3 * HID], BF16))
        h0m = ctx.enter_context(nc.sbuf_tensor("h0m_s", [128, NB * HID], BF16))
        w2f = ctx.enter_context(nc.sbuf_tensor("w2f_s", [128, 2 * HID2], BF16))
        pt = [
            ctx.enter_context(nc.sbuf_tensor(f"ptb{s}", [128, 3 * W], BF16))
            for s in range(PTBUF)
        ]
        at = [
            ctx.enter_context(nc.sbuf_tensor(f"atb{s}", [128, 2 * N], BF16))
            for s in range(3)
        ]
        h2sb = [
            ctx.enter_context(nc.sbuf_tensor(f"h2b{s}", [128, N], BF16))
            for s in range(3)
        ]
        h3sb = [
            ctx.enter_context(nc.sbuf_tensor(f"h3b{s}", [128, W], BF16))
            for s in range(3)
        ]
        srow = [
            ctx.enter_context(nc.sbuf_tensor(f"srowb{s}", [1, N], F32))
            for s in range(SRBUF)
        ]
        # ---- PSUM: 8 banks exactly ----
        aps = [
            [
                ctx.enter_context(nc.psum_tensor(f"apsb{p}{h}", [128, N], F32))
                for h in range(2)
            ]
            for p in range(2)
        ]  # aps[ctx%2][hc]
        h2ps = [
            ctx.enter_context(nc.psum_tensor(f"h2psb{s}", [128, N], F32))
            for s in range(2)
        ]
        h3ps = [
            ctx.enter_context(nc.psum_tensor(f"h3psb{s}", [128, W], F32))
            for s in range(2)
        ]

        # ---- semaphores ----
        sem_const = ctx.enter_context(nc.semaphore("sem_const"))
        sem_h0m = [
            ctx.enter_context(nc.semaphore(f"sem_h0m{p}"))
            for p in range(N_H0M_PIECES)
        ]
        sem_w2 = ctx.enter_context(nc.semaphore("sem_w2"))
        sem_pt = [
            ctx.enter_context(nc.semaphore(f"sem_pt{s}")) for s in range(PTBUF)
        ]
        sem_out = [
            ctx.enter_context(nc.semaphore(f"sem_out{s}")) for s in range(SRBUF)
        ]
        sem_mm1 = ctx.enter_context(nc.semaphore("sem_mm1"))
        sem_relu = ctx.enter_context(nc.semaphore("sem_relu"))
        sem_mm2 = ctx.enter_context(nc.semaphore("sem_mm2"))
        sem_mm3 = ctx.enter_context(nc.semaphore("sem_mm3"))
        sem_mm4 = ctx.enter_context(nc.semaphore("sem_mm4"))
        sem_dve = ctx.enter_context(nc.semaphore("sem_dve"))

        block = ctx.enter_context(nc.Block())

        @block.sync
        def _(sync):
            # piece 0 (slot 0's chunk) first: it gates the very first matmul
            sync.dma_start(h0m[:, 0:HID], h0m_d[:, 0:HID]).then_inc(
                sem_h0m[0], 16
            )
            sync.dma_start(h0f[:, :], h0f_d[:, :]).then_inc(sem_const, 16)
            sync.dma_start(w2f[:, :], w2f_d[:, :]).then_inc(sem_w2, 16)
            sync.dma_start(
                h0m[:, HID : 8 * HID], h0m_d[:, HID : 8 * HID]
            ).then_inc(sem_h0m[1], 16)
            for p in range(1, 6):
                sync.dma_start(
                    h0m[:, 8 * p * HID : 8 * (p + 1) * HID],
                    h0m_d[:, 8 * p * HID : 8 * (p + 1) * HID],
                ).then_inc(sem_h0m[p + 1], 16)
            for i in range(NI):
                k = i - 3
                if 0 <= k < NB:
                    sync.wait_ge(sem_dve, c_sc[k])
                    sync.dma_start(
                        out_ds[k][:, :], srow[k % SRBUF][:, 0 : _jw(k)]
                    ).then_inc(sem_out[k % SRBUF], 16)

        @block.gpsimd
        def _(g):
            for p in range(min(PTBUF, NB)):
                if p >= 2:
                    # keep only 2 prefetch DMAs in flight so pt(0) is not
                    # bandwidth-shared 6 ways (rings interleave packets)
                    g.wait_ge(sem_pt[p - 2], 16)
                g.dma_start(pt[p][:, :], pt_d[p]).then_inc(sem_pt[p], 16)
            for i in range(NI):
                p = i + PTBUF
                if p < NB:
                    g.wait_ge(sem_mm3, i + 1)
                    g.dma_start(
                        pt[p % PTBUF][:, :], pt_d[p]
                    ).then_inc(sem_pt[p % PTBUF], 16)

        @block.tensor
        def _(te):
            te.wait_ge(sem_const, 16)
            for i in range(NI):
                if i == 1:
                    te.wait_ge(sem_w2, 16)
                # ---- mm1(i): A_T = H0m^T-contracted with Pt, F=384 x6 ----
                if i < NB:
                    # aps[i%2] reuse (relu(i-2) drained) is implied by the
                    # previous iteration's sem_relu wait before mm2(i-2).
                    if i == 0 or _h0m_piece(i) != _h0m_piece(i - 1):
                        te.wait_ge(sem_h0m[_h0m_piece(i)], 16)
                    te.wait_ge(sem_pt[i % PTBUF], 16 * (i // PTBUF + 1))
                    ptt = pt[i % PTBUF]
                    tg = i // 16  # chunk whose row g is modified
                    for hc in range(2):
                        for t in range(3):
                            if t == tg:
                                stat = h0m[:, i * HID + hc * 128 : i * HID + hc * 128 + 128]
                            else:
                                stat = h0f[:, t * HID + hc * 128 : t * HID + hc * 128 + 128]
                            mm = nc.tensor.matmul(
                                aps[i % 2][hc][:, :],
                                stat,
                                ptt[:, t * W + 1 : t * W + 1 + N],
                                start=(t == 0),
                                stop=(t == 2),
                            )
                        mm.then_inc(sem_mm1, 1)  # hc group done -> relu hc
                # ---- mm2(i-1): h2 = A@W2, F=128 x6 ----
                k = i - 1
                if 0 <= k < NB:
                    # h2ps[k%2] reuse (DVE h2c(k-2) drained) is implied by the
                    # previous iteration's sem_dve wait before mm3(k-1).
                    te.wait_ge(sem_relu, 2 * k + 2)
                    dst = h2ps[k % 2]
                    for jc in range(3):
                        for ht in range(2):
                            mm = nc.tensor.matmul(
                                dst[:, jc * 128 : (jc + 1) * 128],
                                at[k % 3][
                                    :, ht * N + jc * 128 : ht * N + jc * 128 + 128
                                ],
                                w2f[:, ht * HID2 : (ht + 1) * HID2],
                                start=(ht == 0),
                                stop=(ht == 1),
                            )
                    mm.then_inc(sem_mm2, 1)
                # ---- mm3(i-2): h3T truncated (col 0 = d), F=8k+10 x3 ----
                k = i - 2
                if 0 <= k < NB:
                    fw = _jw(k) + 2  # prow col + j cols (8k+8) + pad
                    # one wait covers h2c(k) and, via DVE ordering, the
                    # S-row drain sc(k-2) of the aliased h3ps[k%2] row 0
                    # plus h3c/h2c buffer reuse.
                    te.wait_ge(sem_dve, c_sc[k - 2] if k >= 2 else c_h2c[k])
                    dst = h3ps[k % 2]
                    ptt = pt[k % PTBUF]
                    for t in range(3):
                        mm = nc.tensor.matmul(
                            dst[:, 0:fw],
                            h2sb[k % 3][:, t * 128 : (t + 1) * 128],
                            ptt[:, t * W : t * W + fw],
                            start=(t == 0),
                            stop=(t == 2),
                        )
                    mm.then_inc(sem_mm3, 1)
                # ---- mm4(i-3): S row into h3ps[k%2] partition 0, F=8k+8 ----
                k = i - 3
                if 0 <= k < NB:
                    if k == 0 or k == NB - 1:
                        # k=0: mm3(1)'s c_h2c[1] wait does not cover h3c(0)
                        # (h3c follows h2c within a DVE iteration);
                        # k=NB-1: no mm3 in this iteration to carry the wait.
                        te.wait_ge(sem_dve, c_h3c[k])
                    mm = nc.tensor.matmul(
                        h3ps[k % 2][0:1, 0 : _jw(k)],
                        h3sb[k % 3][:, 0:1],
                        h3sb[k % 3][:, 1 : 1 + _jw(k)],
                        start=True,
                        stop=True,
                    )
                    mm.then_inc(sem_mm4, 1)

        @block.scalar
        def _(sc):
            for i in range(NI):
                k = i
                if k < NB:
                    if k >= 3:
                        sc.wait_ge(sem_mm2, k - 2)  # at[k%3] reuse
                    for hc in range(2):
                        sc.wait_ge(sem_mm1, 2 * k + hc + 1)
                        nc.scalar.activation(
                            at[k % 3][:, hc * N : (hc + 1) * N],
                            aps[k % 2][hc][:, :],
                            AFT.Relu,
                        ).then_inc(sem_relu, 1)

        @block.vector
        def _(ve):
            for i in range(NB + 4):
                k = i - 4
                if 0 <= k < NB:
                    # S-row drain first: frees h3ps[k%2] row 0 for mm3(k+2)
                    # in the same PE iteration.
                    ve.wait_ge(sem_mm4, k + 1)
                    if k >= SRBUF:
                        ve.wait_ge(sem_out[k % SRBUF], 16 * (k // SRBUF))
                    nc.vector.tensor_copy(
                        srow[k % SRBUF][:, 0 : _jw(k)],
                        h3ps[k % 2][0:1, 0 : _jw(k)],
                    ).then_inc(sem_dve, 1)
                k = i - 1
                if 0 <= k < NB:
                    # h2sb[k%3] reuse (mm3(k-3) done) is implied by the
                    # previous iteration's sem_mm3 wait before h3c(k-1).
                    ve.wait_ge(sem_mm2, k + 1)
                    nc.vector.tensor_copy(
                        h2sb[k % 3][:, :], h2ps[k % 2][:, :]
                    ).then_inc(sem_dve, 1)
                k = i - 2
                if 0 <= k < NB:
                    if k >= 3:
                        ve.wait_ge(sem_mm4, k - 2)  # h3sb[k%3] reuse
                    ve.wait_ge(sem_mm3, k + 1)
                    nc.vector.tensor_copy(
                        h3sb[k % 3][:, 0 : _jw(k) + 2],
                        h3ps[k % 2][:, 0 : _jw(k) + 2],
                    ).then_inc(sem_dve, 1)

    return nc


def _get_nc() -> bass.Bass:
    if "nc" not in _NC_CACHE:
        _NC_CACHE["nc"] = _build_nc()
    return _NC_CACHE["nc"]


def kernel(z, x, partials, W1, W2):
    global LAST_RESULT
    z = np.asarray(z, dtype=np.float32)
    x = np.asarray(x, dtype=np.float32)
    partials = np.asarray(partials, dtype=np.float32)
    W1 = np.asarray(W1, dtype=np.float32)
    W2 = np.asarray(W2, dtype=np.float32)

    H0 = z[0] @ W1[:D]  # [384, 256]
    h0f = (
        np.ascontiguousarray(H0.reshape(3, 128, HID).transpose(1, 0, 2))
        .reshape(128, 3 * HID)
        .astype(ml_dtypes.bfloat16)
    )
    w1r = W1[D]  # [256]
    w2f = (
        np.ascontiguousarray(W2.reshape(2, 128, HID2).transpose(1, 0, 2))
        .reshape(128, 2 * HID2)
        .astype(ml_dtypes.bfloat16)
    )

    ptT = np.ascontiguousarray(partials.transpose(0, 2, 1))  # ptT[g,j,i]=P_g[i,j]
    ar = np.arange(N)
    prow = partials[ar, ar, :]  # [384, 384]  P_g[g, :]

    in_maps = []
    for c in range(NCORES):
        gs = np.arange(NB) * NCORES + c  # slot b -> context g = 8b + c
        aug = np.zeros((NB, 3, 128, W), dtype=ml_dtypes.bfloat16)
        aug[..., 1 : 1 + N] = (
            ptT[gs].reshape(NB, 3, 128, N).astype(ml_dtypes.bfloat16)
        )
        aug[..., 0] = prow[gs].reshape(NB, 3, 128).astype(ml_dtypes.bfloat16)
        aug = np.ascontiguousarray(aug.transpose(0, 2, 1, 3)).reshape(
            NB, 128, 3 * W
        )
        # per-slot modified stationary chunk: H0's chunk b//16 with row g += W1r
        h0m = np.empty((NB, 128, HID), dtype=np.float32)
        for b in range(NB):
            g = 8 * b + c
            t = b // 16
            h0m[b] = H0[t * 128 : (t + 1) * 128]
            h0m[b, g - t * 128] += w1r
        h0m = (
            np.ascontiguousarray(h0m.transpose(1, 0, 2))
            .reshape(128, NB * HID)
            .astype(ml_dtypes.bfloat16)
        )
        in_maps.append({"pt": aug, "h0f": h0f, "h0m": h0m, "w2f": w2f})

    nc = _get_nc()
    res = run_bass_kernel_spmd(
        nc,
        in_maps,
        core_ids=list(range(NCORES)),
        trace=os.environ.get("KERNEL_TRACE", "0") not in ("0", ""),
    )
    LAST_RESULT = res
    S = np.zeros((N, N), dtype=np.float32)
    for c in range(NCORES):
        for b in range(NB):
            S[8 * b + c, 0 : _jw(b)] = np.asarray(
                res.results[c][f"o{b:02d}"], np.float32
            )[0]
    sup = np.tril(S)
    sup = (sup + sup.T) * np.float32(0.5)
    return (x + sup).astype(np.float32)
